# revision 25
# baseline (speedup 1.0000x reference)
"""Trainium2 Bass kernel for nn_NonLocalLayer (8-core data-parallel).

Math per batch n (see reference):
  theta = st @ w_st + b_st        (256,128)  -> reinterpret (128,256)  "theta_r"
  phi   = lt @ w_lt + b_lt        (4096,128) -> reinterpret (128,4096) "phi_r"
  g     = lt @ w_g  + b_g         (4096,128) -> reinterpret (128,4096) "g_r"
  attn  = theta_r^T @ phi_r / sqrt(128); p = softmax(attn, axis=l)
  out2  = g_r @ p^T               (128,256)
  y     = relu(LN(out2) * gamma + beta)      (128,256)
  out   = y[:, :, None]*w_out + b_out        (128,256,512)

Device strategy (per core = one batch):
  - host pre-transposes AND column-permutes st/lt (ltTP[c, m*128+i] =
    ltT[c, 32*i+m]) so every phi_r/g_r block is a contiguous matmul
  - big matmuls in fp16 (1 cyc/row on PE); accumulation stays fp32 in PSUM
  - softmax in transposed orientation (l on partitions) without
    max-subtraction (attn bounded ~ +-8); sums via ones-matmul (out2 into
    two alternating PSUM banks); normalization + LayerNorm folded into a
    short fused scalar_tensor_tensor chain with accum_out row-sums
  - epilogue: output stored TRANSPOSED as outT[k, c*256+s] in fp16.
    y (fp16) bounces through a 64KB DRAM buffer onto one partition row;
    the idle PE replicates it to all 128 partitions (ones ⊗ yrow into
    PSUM), ACT copies PSUM->fp16, and outT[k,:] = w[k]*y + b[k] is one
    fused per-partition-scalar DVE op per (kblock, chunk). No PE rank-2
    spam, no HBM broadcast reads stealing write bandwidth. Host
    un-transposes (cheap numpy).
"""
import math
import os

import numpy as np

NB = 8          # batch == n cores
S = 256         # NUM_ST
L = 4096        # NUM_LT
C = 512         # C_ST == C_LT
D = 128         # C_LAT
INV_SQRT_D = 1.0 / math.sqrt(float(D))
LN_EPS = 1e-3
J = D * S       # 32768 flattened (c,s) -> j = c*256 + s
# epilogue chunk schedule in 512-col units: small chunks at both ends
# (fast pipeline fill, short drain tail), big in the middle
SIZES5 = [2, 4, 8, 8, 8, 8, 8, 8, 4, 4, 2]
assert sum(SIZES5) * 512 == J

_CACHE = {}
LAST_EXEC_NS = None


def _build_program():
    import concourse.bacc as bacc
    import concourse.bass as bass
    import concourse.tile as tile
    from concourse import mybir

    dt = mybir.dt
    F32 = dt.float32
    F16 = dt.float16
    AF = mybir.ActivationFunctionType
    OP = mybir.AluOpType

    nc = bacc.Bacc("TRN2", target_bir_lowering=False, debug=False,
                   num_devices=NB)

    lin_dt = F16
    d_ltT = nc.dram_tensor("ltT", [C, L], lin_dt, kind="ExternalInput")
    # weights packed so each loads as ONE dma: [c_block(128), j*128 + d]
    d_stA = nc.dram_tensor("stT", [128, 4 * S], lin_dt, kind="ExternalInput")
    d_wst = nc.dram_tensor("wst", [128, 4 * D], lin_dt, kind="ExternalInput")
    d_wlt = nc.dram_tensor("wlt", [128, 4 * D], lin_dt, kind="ExternalInput")
    d_wg = nc.dram_tensor("wg", [128, 4 * D], lin_dt, kind="ExternalInput")
    d_bst = nc.dram_tensor("bst", [1, D], F16, kind="ExternalInput")
    d_blt = nc.dram_tensor("blt", [D, 1], F32, kind="ExternalInput")
    d_bg = nc.dram_tensor("bg", [D, 1], F32, kind="ExternalInput")
    d_gam = nc.dram_tensor("gam", [D, S], F32, kind="ExternalInput")
    d_bet = nc.dram_tensor("bet", [D, S], F32, kind="ExternalInput")
    d_idh = nc.dram_tensor("identh", [128, 128], F16, kind="ExternalInput")
    d_wk = nc.dram_tensor("wk", [128, 4], F32, kind="ExternalInput")
    d_bk = nc.dram_tensor("bk", [128, 4], F32, kind="ExternalInput")
    # y bounce buffer (read back to one partition row) + transposed output
    d_y = nc.dram_tensor("ybounce", [D, S], F16, kind="ExternalOutput")
    d_out = nc.dram_tensor("out", [C, J], F16, kind="ExternalOutput")

    with tile.TileContext(nc) as tc:
        # ---------- persistent pool (lives whole kernel) ----------
        with tc.tile_pool(name="keep", bufs=1) as keep:
            identh = keep.tile([128, 128], F16, tag="identh")
            bsth = keep.tile([1, D], F16, tag="bsth")
            blt_c = keep.tile([D, 1], F32, tag="blt_c")
            bg_c = keep.tile([D, 1], F32, tag="bg_c")
            gam = keep.tile([D, S], F32, tag="gam")
            bet = keep.tile([D, S], F32, tag="bet")
            wk = keep.tile([128, 4], F32, tag="wk")
            bk = keep.tile([128, 4], F32, tag="bk")

            ones_f = keep.tile([128, 1], F32, tag="ones_f")
            nc.vector.memset(ones_f[:], 1.0)
            ones_r = keep.tile([128, 1], F16, tag="ones_r")
            nc.vector.tensor_copy(ones_r[:], ones_f[:])
            orow_f = keep.tile([1, 128], F32, tag="orow_f")
            nc.vector.memset(orow_f[:], 1.0)
            orow_h = keep.tile([1, 256], F16, tag="orow_h")
            nc.vector.memset(orow_h[:], 1.0)

            theta_r = keep.tile([128, S], F16, tag="theta_r")
            y_h = keep.tile([D, S], F16, tag="y_h")
            yrow = keep.tile([1, J], F16, tag="yrow")

            # ---------- main phase ----------
            with tc.tile_pool(name="main", bufs=1) as main:
                # ltTP: host-permuted so phi/g blocks are contiguous slices
                ltTP = [main.tile([128, L], F16, tag=f"ltT{j}", name=f"ltT{j}")
                        for j in range(4)]
                stA = main.tile([128, 4 * S], F16, tag="stA")
                wstA = main.tile([128, 4 * D], F16, tag="wstA")
                wltA = main.tile([128, 4 * D], F16, tag="wltA")
                wgA = main.tile([128, 4 * D], F16, tag="wgA")

                engs = [nc.gpsimd, nc.sync, nc.scalar]

                def ltq(j, t, eng):  # quarter-column loads (1024 cols, 256KB)
                    eng.dma_start(
                        ltTP[j][:, 1024 * t:1024 * (t + 1)],
                        d_ltT[128 * j:128 * (j + 1), 1024 * t:1024 * (t + 1)])

                # phi/g weights + first lt quarter first (first slice only
                # needs cols 0:512) on the low-latency HWDGE queues
                # (sync/scalar); gpsimd's software DGE has ~4us latency so it
                # only gets non-critical later quarters and small constants.
                nc.sync.dma_start(wltA[:], d_wlt[:])
                nc.scalar.dma_start(wgA[:], d_wg[:])
                ltq(0, 0, nc.sync)
                ltq(1, 0, nc.scalar)
                ltq(2, 0, nc.sync)
                ltq(3, 0, nc.scalar)
                nc.scalar.dma_start(identh[:], d_idh[:])
                nc.gpsimd.dma_start(bsth[:], d_bst[:])
                nc.gpsimd.dma_start(blt_c[:], d_blt[:])
                nc.gpsimd.dma_start(bg_c[:], d_bg[:])
                nc.sync.dma_start(stA[:], d_stA[:])
                nc.scalar.dma_start(wstA[:], d_wst[:])
                ke = 0
                for t in (1, 2, 3):
                    for j in range(4):
                        ltq(j, t, engs[ke % 3]); ke += 1
                nc.scalar.dma_start(gam[:], d_gam[:])
                nc.scalar.dma_start(bet[:], d_bet[:])
                nc.gpsimd.dma_start(wk[:], d_wk[:])
                nc.gpsimd.dma_start(bk[:], d_bk[:])

                # phiTP / gTP in permuted-column order, fp16, built slicewise;
                # attention loop pipelined against slice production
                phiP = main.tile([D, L], F16, tag="phiP")
                gP = main.tile([D, L], F16, tag="gP")

                u = main.tile([D, S], F32, tag="u")
                sums_sb = main.tile([1, S], F32, tag="sums_sb")

                with tc.tile_pool(name="psL", bufs=1, space="PSUM") as psL, \
                     tc.tile_pool(name="loop", bufs=1) as lp:
                    # two alternating accumulator banks for out2 (breaks the
                    # back-to-back same-bank accumulate stall); separate banks
                    # because a PSUM zero-region admits only one open group
                    p_acc = [psL.tile([D, S], F32, tag=f"acc{i}",
                                      name=f"acc{i}") for i in range(2)]
                    p_sums = psL.tile([1, S], F32, tag="sums")

                    def emit_theta():
                        for h in range(2):
                            pth = psL.tile([128, D], F32, tag="att2", bufs=2,
                                           name=f"pth{h}")
                            for j in range(4):
                                nc.tensor.matmul(
                                    pth[:],
                                    stA[:, 256 * j + 128 * h:
                                        256 * j + 128 * (h + 1)],
                                    wstA[:, 128 * j:128 * (j + 1)],
                                    start=(j == 0), stop=False)
                            nc.tensor.matmul(pth[:], orow_h[:, 0:128],
                                             bsth[:], start=False, stop=True)
                            nc.vector.tensor_copy(
                                theta_r[:, 128 * h:128 * (h + 1)], pth[:])

                    def emit_slice(sl):
                        cols = slice(512 * sl, 512 * (sl + 1))
                        for dst, wts, bias_t in ((phiP, wltA, blt_c),
                                                 (gP, wgA, bg_c)):
                            pmm = psL.tile([D, 512], F32, tag="mm", bufs=2,
                                           name=f"pmm{sl}")
                            for j in range(4):
                                nc.tensor.matmul(
                                    pmm[:], wts[:, 128 * j:128 * (j + 1)],
                                    ltTP[j][:, cols],
                                    start=(j == 0), stop=(j == 3))
                            nc.scalar.activation(dst[:, cols], pmm[:],
                                                 AF.Identity,
                                                 bias=bias_t[:, 0:1])

                    ers = {}
                    phiRs = {}
                    for it in range(35):
                        if it % 4 == 0 and it // 4 < 8:
                            emit_slice(it // 4)
                        if it == 1:
                            emit_theta()
                        # stage A: transpose phi block m (contiguous now)
                        if it < 32:
                            m = it
                            ptp = psL.tile([128, 128], F16, tag="ptp", bufs=1,
                                           name=f"ptp{m}")
                            nc.tensor.transpose(
                                ptp[:], phiP[:, 128 * m:128 * (m + 1)],
                                identh[:])
                            phiR = lp.tile([128, 128], F16, tag="phiR", bufs=4,
                                           name=f"phiR{m}")
                            nc.vector.tensor_copy(phiR[:], ptp[:])
                            phiRs[m] = phiR
                        # stage B: attn matmul + exp
                        if 1 <= it <= 32:
                            m = it - 1
                            p_att = psL.tile([128, S], F32, tag="att2", bufs=2,
                                             name=f"patt{m}")
                            nc.tensor.matmul(p_att[:], phiRs.pop(m)[:],
                                             theta_r[:], start=True, stop=True)
                            er = lp.tile([128, S], F16, tag="er", bufs=4,
                                         name=f"er{m}")
                            nc.scalar.activation(er[:], p_att[:], AF.Exp,
                                                 scale=INV_SQRT_D)
                            ers[m] = er
                        # stage C: accumulate out2 (alternating banks) + sums
                        if it >= 3:
                            m = it - 3
                            er = ers.pop(m)
                            nc.tensor.matmul(p_acc[m % 2][:],
                                             gP[:, 128 * m:128 * (m + 1)],
                                             er[:], start=(m < 2),
                                             stop=(m >= 30))
                            nc.tensor.matmul(p_sums[:], ones_r[:], er[:],
                                             start=(m == 0), stop=(m == 31))

                    # merge banks (only one PSUM operand allowed per op)
                    uh = main.tile([D, S], F32, tag="uh")
                    nc.vector.tensor_copy(uh[:], p_acc[1][:])
                    nc.vector.tensor_tensor(u[:], p_acc[0][:], uh[:],
                                            OP.add)
                    nc.vector.tensor_copy(sums_sb[:], p_sums[:])

                # ---------- softmax-normalize + LayerNorm + ReLU ----------
                with tc.tile_pool(name="psN", bufs=1, space="PSUM") as psN:
                    # 1/sums (fast approx, ~18 bits), broadcast via PE, then
                    # one fused normalize that also emits row-sums (mean)
                    rec = main.tile([1, S], F32, tag="rec")
                    nc.vector.reciprocal_approx_fast(rec[:], sums_sb[:])
                    p_rbS = psN.tile([128, S], F32, tag="rb")
                    nc.tensor.matmul(p_rbS[:], orow_f[:], rec[:],
                                     start=True, stop=True)
                    acc2 = main.tile([128, 2], F32, tag="acc2")
                    out2 = main.tile([D, S], F32, tag="out2")
                    nc.vector.scalar_tensor_tensor(
                        out2[:], u[:], 1.0, p_rbS[:], OP.mult, OP.mult,
                        accum_out=acc2[:, 0:1])
                    sqj = main.tile([D, S], F32, tag="sqj")
                    nc.vector.scalar_tensor_tensor(
                        sqj[:], out2[:], 1.0, out2[:], OP.mult, OP.mult,
                        accum_out=acc2[:, 1:2])
                    p_st = psN.tile([1, 2], F32, tag="st")
                    nc.tensor.matmul(p_st[:], ones_f[:], acc2[:],
                                     start=True, stop=True)
                    stat = main.tile([1, 4], F32, tag="stat")
                    # mean, e2
                    nc.vector.tensor_scalar(stat[:, 0:2], p_st[:],
                                            1.0 / (D * S), None, OP.mult)
                    # var = e2 - mean^2 ; vare = var + eps
                    nc.vector.tensor_tensor(stat[:, 2:3], stat[:, 0:1],
                                            stat[:, 0:1], OP.mult)
                    nc.vector.tensor_tensor(stat[:, 3:4], stat[:, 1:2],
                                            stat[:, 2:3], OP.subtract)
                    vare = main.tile([1, 1], F32, tag="vare")
                    nc.vector.tensor_scalar(vare[:], stat[:, 3:4], LN_EPS,
                                            None, OP.add)
                    sqv = main.tile([1, 1], F32, tag="sqv")
                    nc.scalar.activation(sqv[:], vare[:], AF.Sqrt)
                    ms = main.tile([1, 2], F32, tag="ms")
                    nc.vector.tensor_copy(ms[:, 0:1], stat[:, 0:1])
                    nc.vector.reciprocal(ms[:, 1:2], sqv[:])
                    p_ms = psN.tile([128, 2], F32, tag="ms2")
                    nc.tensor.matmul(p_ms[:], orow_f[:], ms[:],
                                     start=True, stop=True)
                    # y = relu(((out2 - m) * gamma) * r + beta), fp16
                    t2p = main.tile([D, S], F32, tag="t2p")
                    nc.vector.scalar_tensor_tensor(
                        t2p[:], out2[:], p_ms[:, 0:1], gam[:],
                        OP.subtract, OP.mult)
                    t3 = main.tile([D, S], F32, tag="t3")
                    nc.vector.scalar_tensor_tensor(
                        t3[:], t2p[:], p_ms[:, 1:2], bet[:],
                        OP.mult, OP.add)
                    nc.vector.tensor_scalar_max(y_h[:], t3[:], 0.0)

            # ---------- epilogue: outT[k, c*256+s] = w[k]*y + b[k] ----------
            # y -> DRAM (64KB) -> back as one partition row; idle PE
            # replicates it to 128 partitions; DVE does the fused mult-add.
            nc.sync.dma_start(d_y[:, :], y_h[:])
            ybase = d_y[:, :]
            import concourse.bass as bass_mod
            yr_src = bass_mod.AP(tensor=ybase.tensor, offset=ybase.offset,
                                 ap=[[J, 1], [1, J]])
            nc.sync.dma_start(yrow[0:1, :], yr_src)
            with tc.tile_pool(name="epi", bufs=1) as ep, \
                 tc.tile_pool(name="psE", bufs=1, space="PSUM") as psE:
                off = 0
                for ci, n5 in enumerate(SIZES5):
                    sz = 512 * n5
                    ybg = ep.tile([128, 4096], F16, tag="ybg", bufs=2,
                                  name=f"ybg{ci}")
                    for t in range(n5):
                        pb = psE.tile([128, 512], F32, tag="pb", bufs=4,
                                      name=f"pb{ci}_{t}")
                        nc.tensor.matmul(
                            pb[:], orow_h[:, 0:128],
                            yrow[0:1, off + 512 * t:off + 512 * (t + 1)],
                            start=True, stop=True)
                        nc.scalar.activation(ybg[:, 512 * t:512 * (t + 1)],
                                             pb[:], AF.Identity)
                    for kb in range(4):
                        oc = ep.tile([128, 4096], F16, tag="oc", bufs=6,
                                     name=f"oc{ci}_{kb}")
                        nc.vector.tensor_scalar(
                            oc[:, 0:sz], ybg[:, 0:sz], wk[:, kb:kb + 1],
                            bk[:, kb:kb + 1], OP.mult, OP.add)
                        nc.sync.dma_start(
                            d_out[128 * kb:128 * (kb + 1), off:off + sz],
                            oc[:, 0:sz])
                    off += sz

    nc.compile()
    return nc


def _get_program():
    if "nc" not in _CACHE:
        _CACHE["nc"] = _build_program()
    return _CACHE["nc"]


def _install_ntff_shim():
    """Provide antenv.axon_hooks (absent in this image) so trace=True can
    capture NTFF profiles through the axon .so. Best-effort."""
    import sys
    import types
    try:
        from antenv.axon_hooks import get_axon_ntff_profile_hook  # noqa
        return
    except ImportError:
        pass
    try:
        from trn_agent_boot.trn_boot import _ntff_profile_via_ctypes
        hook = _ntff_profile_via_ctypes("/opt/axon/libaxon_pjrt.so")
        mod = types.ModuleType("antenv.axon_hooks")
        state = {"h": hook}
        mod.set_axon_ntff_profile_hook = lambda h: state.__setitem__("h", h)
        mod.get_axon_ntff_profile_hook = lambda: state["h"]
        sys.modules["antenv.axon_hooks"] = mod
        import antenv
        antenv.axon_hooks = mod
    except Exception as e:  # profiling is optional
        print(f"ntff shim unavailable: {e}")


def kernel(st_feat, lt_feat, w_st, b_st, w_lt, b_lt, w_g, b_g,
           ln_gamma, ln_beta, w_out, b_out):
    from concourse.bass_utils import run_bass_kernel_spmd
    global LAST_EXEC_NS

    st_feat = np.asarray(st_feat, dtype=np.float32)
    lt_feat = np.asarray(lt_feat, dtype=np.float32)

    def pack4(a):  # (512, X) -> (128, 4*X) with block j at cols [X*j, X*j+X)
        x = a.shape[1]
        return np.ascontiguousarray(
            a.reshape(4, 128, x).transpose(1, 0, 2).reshape(128, 4 * x))

    wst = pack4(np.asarray(w_st, np.float32).astype(np.float16))
    wlt = pack4(np.asarray(w_lt, np.float32).astype(np.float16))
    wg = pack4(np.asarray(w_g, np.float32).astype(np.float16))
    gam = np.ascontiguousarray(np.asarray(ln_gamma, np.float32)
                               .reshape(D, S))
    bet = np.ascontiguousarray(np.asarray(ln_beta, np.float32).reshape(D, S))
    bstv = np.asarray(b_st, np.float32).astype(np.float16).reshape(1, D)
    bltv = np.asarray(b_lt, np.float32).reshape(D, 1)
    bgv = np.asarray(b_g, np.float32).reshape(D, 1)
    identh = np.eye(128, dtype=np.float16)
    wkv = np.ascontiguousarray(
        np.asarray(w_out, np.float32).reshape(4, 128).T)
    bkv = np.ascontiguousarray(
        np.asarray(b_out, np.float32).reshape(4, 128).T)

    in_maps = []
    for n in range(NB):
        # column-permuted transposes: ltTP[c, m*128 + i] = ltT[c, 32*i + m]
        # and stTP[c, h*128 + i] = stT[c, 2*i + h]
        ltT = lt_feat[n].reshape(L, C).T.astype(np.float16)
        ltTP = np.ascontiguousarray(
            ltT.reshape(C, 128, 32).transpose(0, 2, 1).reshape(C, L))
        stT = st_feat[n].reshape(S, C).T.astype(np.float16)
        stTP = pack4(np.ascontiguousarray(
            stT.reshape(C, 128, 2).transpose(0, 2, 1).reshape(C, S)))
        in_maps.append({
            "ltT": ltTP, "stT": stTP, "wst": wst, "wlt": wlt, "wg": wg,
            "bst": bstv, "blt": bltv, "bg": bgv,
            "gam": gam, "bet": bet, "identh": identh,
            "wk": wkv, "bk": bkv,
        })

    nc = _get_program()
    trace = os.environ.get("BASS_KERNEL_TRACE", "") == "1"
    if trace:
        _install_ntff_shim()
    res = run_bass_kernel_spmd(nc, in_maps, core_ids=list(range(NB)),
                               trace=trace)
    LAST_EXEC_NS = res.exec_time_ns
    out = np.empty((NB, D, S, 1, C), np.float32)
    for n in range(NB):
        r = np.asarray(res.results[n]["out"])  # (512, 32768) fp16
        out[n] = (r.reshape(C, D, S).transpose(1, 2, 0)
                  .astype(np.float32).reshape(D, S, 1, C))
    return out


# revision 27
# speedup vs baseline: 1.1022x; 1.1022x over previous
"""Trainium2 Bass kernel for nn_NonLocalLayer (8-core data-parallel).

Math per batch n (see reference):
  theta = st @ w_st + b_st        (256,128)  -> reinterpret (128,256)  "theta_r"
  phi   = lt @ w_lt + b_lt        (4096,128) -> reinterpret (128,4096) "phi_r"
  g     = lt @ w_g  + b_g         (4096,128) -> reinterpret (128,4096) "g_r"
  attn  = theta_r^T @ phi_r / sqrt(128); p = softmax(attn, axis=l)
  out2  = g_r @ p^T               (128,256)
  y     = relu(LN(out2) * gamma + beta)      (128,256)
  out   = y[:, :, None]*w_out + b_out        (128,256,512)

Device strategy (per core = one batch):
  - host pre-transposes AND column-permutes st/lt (ltTP[c, m*128+i] =
    ltT[c, 32*i+m]) so every phi_r/g_r block is a contiguous matmul
  - big matmuls in fp16 (1 cyc/row on PE); accumulation stays fp32 in PSUM
  - softmax in transposed orientation (l on partitions) without
    max-subtraction (attn bounded ~ +-8); sums via ones-matmul (out2 into
    two alternating PSUM banks); normalization + LayerNorm folded into a
    short fused scalar_tensor_tensor chain with accum_out row-sums
  - epilogue: output stored TRANSPOSED as outT[k, c*256+s] in fp16.
    y (fp16) bounces through a 64KB DRAM buffer onto one partition row;
    the idle PE replicates it to all 128 partitions (ones ⊗ yrow into
    PSUM), ACT copies PSUM->fp16, and outT[k,:] = w[k]*y + b[k] is one
    fused per-partition-scalar DVE op per (kblock, chunk). No PE rank-2
    spam, no HBM broadcast reads stealing write bandwidth. Host
    un-transposes (cheap numpy).
"""
import math
import os

import numpy as np

NB = 8          # batch == n cores
S = 256         # NUM_ST
L = 4096        # NUM_LT
C = 512         # C_ST == C_LT
D = 128         # C_LAT
INV_SQRT_D = 1.0 / math.sqrt(float(D))
LN_EPS = 1e-3
J = D * S       # 32768 flattened (c,s) -> j = c*256 + s
# epilogue chunk schedule in 512-col units: small chunks at both ends
# (fast pipeline fill, short drain tail), big in the middle
SIZES5 = [2, 4, 8, 8, 8, 8, 8, 8, 4, 4, 2]
assert sum(SIZES5) * 512 == J

_CACHE = {}
LAST_EXEC_NS = None


def _build_program():
    import concourse.bacc as bacc
    import concourse.bass as bass
    import concourse.tile as tile
    from concourse import mybir

    dt = mybir.dt
    F32 = dt.float32
    F16 = dt.float16
    AF = mybir.ActivationFunctionType
    OP = mybir.AluOpType

    nc = bacc.Bacc("TRN2", target_bir_lowering=False, debug=False,
                   num_devices=NB)

    lin_dt = F16
    d_ltT = nc.dram_tensor("ltT", [C, L], lin_dt, kind="ExternalInput")
    # weights packed so each loads as ONE dma: [c_block(128), j*128 + d]
    d_stA = nc.dram_tensor("stT", [128, 4 * S], lin_dt, kind="ExternalInput")
    d_wst = nc.dram_tensor("wst", [128, 4 * D], lin_dt, kind="ExternalInput")
    d_wlt = nc.dram_tensor("wlt", [128, 4 * D], lin_dt, kind="ExternalInput")
    d_wg = nc.dram_tensor("wg", [128, 4 * D], lin_dt, kind="ExternalInput")
    d_bst = nc.dram_tensor("bst", [1, D], F16, kind="ExternalInput")
    d_blt = nc.dram_tensor("blt", [D, 1], F32, kind="ExternalInput")
    d_bg = nc.dram_tensor("bg", [D, 1], F32, kind="ExternalInput")
    d_gam = nc.dram_tensor("gam", [D, S], F32, kind="ExternalInput")
    d_bet = nc.dram_tensor("bet", [D, S], F32, kind="ExternalInput")
    d_idh = nc.dram_tensor("identh", [128, 128], F16, kind="ExternalInput")
    d_wk = nc.dram_tensor("wk", [128, 4], F32, kind="ExternalInput")
    d_bk = nc.dram_tensor("bk", [128, 4], F32, kind="ExternalInput")
    # y bounce buffer (read back to one partition row) + transposed output
    d_y = nc.dram_tensor("ybounce", [D, S], F16, kind="ExternalOutput")
    d_out = nc.dram_tensor("out", [C, J], F16, kind="ExternalOutput")

    with tile.TileContext(nc) as tc:
        # ---------- persistent pool (lives whole kernel) ----------
        with tc.tile_pool(name="keep", bufs=1) as keep:
            identh = keep.tile([128, 128], F16, tag="identh")
            bsth = keep.tile([1, D], F16, tag="bsth")
            blt_c = keep.tile([D, 1], F32, tag="blt_c")
            bg_c = keep.tile([D, 1], F32, tag="bg_c")
            gam = keep.tile([D, S], F32, tag="gam")
            bet = keep.tile([D, S], F32, tag="bet")
            wk = keep.tile([128, 4], F32, tag="wk")
            bk = keep.tile([128, 4], F32, tag="bk")

            ones_f = keep.tile([128, 1], F32, tag="ones_f")
            nc.vector.memset(ones_f[:], 1.0)
            ones_r = keep.tile([128, 1], F16, tag="ones_r")
            nc.vector.tensor_copy(ones_r[:], ones_f[:])
            orow_f = keep.tile([1, 128], F32, tag="orow_f")
            nc.vector.memset(orow_f[:], 1.0)
            orow_h = keep.tile([1, 256], F16, tag="orow_h")
            nc.vector.memset(orow_h[:], 1.0)

            theta_r = keep.tile([128, S], F16, tag="theta_r")
            y_h = keep.tile([D, S], F16, tag="y_h")
            yrow = keep.tile([1, J], F16, tag="yrow")

            # ---------- main phase ----------
            with tc.tile_pool(name="main", bufs=1) as main:
                # ltTP: host-permuted so phi/g blocks are contiguous slices
                ltTP = [main.tile([128, L], F16, tag=f"ltT{j}", name=f"ltT{j}")
                        for j in range(4)]
                stA = main.tile([128, 4 * S], F16, tag="stA")
                wstA = main.tile([128, 4 * D], F16, tag="wstA")
                wltA = main.tile([128, 4 * D], F16, tag="wltA")
                wgA = main.tile([128, 4 * D], F16, tag="wgA")

                engs = [nc.gpsimd, nc.sync, nc.scalar]

                def ltq(j, t, eng):  # quarter-column loads (1024 cols, 256KB)
                    eng.dma_start(
                        ltTP[j][:, 1024 * t:1024 * (t + 1)],
                        d_ltT[128 * j:128 * (j + 1), 1024 * t:1024 * (t + 1)])

                # phi/g weights + first lt quarter first (first slice only
                # needs cols 0:512) on the low-latency HWDGE queues
                # (sync/scalar); gpsimd's software DGE has ~4us latency so it
                # only gets non-critical later quarters and small constants.
                nc.sync.dma_start(wltA[:], d_wlt[:])
                nc.scalar.dma_start(wgA[:], d_wg[:])
                ltq(0, 0, nc.sync)
                ltq(1, 0, nc.scalar)
                ltq(2, 0, nc.sync)
                ltq(3, 0, nc.scalar)
                nc.scalar.dma_start(identh[:], d_idh[:])
                nc.gpsimd.dma_start(bsth[:], d_bst[:])
                nc.gpsimd.dma_start(blt_c[:], d_blt[:])
                nc.gpsimd.dma_start(bg_c[:], d_bg[:])
                nc.sync.dma_start(stA[:], d_stA[:])
                nc.scalar.dma_start(wstA[:], d_wst[:])
                ke = 0
                for t in (1, 2, 3):
                    for j in range(4):
                        ltq(j, t, engs[ke % 3]); ke += 1
                nc.scalar.dma_start(gam[:], d_gam[:])
                nc.scalar.dma_start(bet[:], d_bet[:])
                nc.gpsimd.dma_start(wk[:], d_wk[:])
                nc.gpsimd.dma_start(bk[:], d_bk[:])

                # phiTP / gTP in permuted-column order, fp16, built slicewise;
                # attention loop pipelined against slice production
                phiP = main.tile([D, L], F16, tag="phiP")
                gP = main.tile([D, L], F16, tag="gP")

                u = main.tile([D, S], F32, tag="u")
                sums_sb = main.tile([1, S], F32, tag="sums_sb")

                with tc.tile_pool(name="psL", bufs=1, space="PSUM") as psL, \
                     tc.tile_pool(name="loop", bufs=1) as lp:
                    # two alternating accumulator banks for out2 (breaks the
                    # back-to-back same-bank accumulate stall); separate banks
                    # because a PSUM zero-region admits only one open group
                    p_acc = [psL.tile([D, S], F32, tag=f"acc{i}",
                                      name=f"acc{i}") for i in range(2)]
                    p_sums = psL.tile([1, S], F32, tag="sums")

                    def emit_theta():
                        for h in range(2):
                            pth = psL.tile([128, D], F32, tag="att2", bufs=2,
                                           name=f"pth{h}")
                            for j in range(4):
                                nc.tensor.matmul(
                                    pth[:],
                                    stA[:, 256 * j + 128 * h:
                                        256 * j + 128 * (h + 1)],
                                    wstA[:, 128 * j:128 * (j + 1)],
                                    start=(j == 0), stop=False)
                            nc.tensor.matmul(pth[:], orow_h[:, 0:128],
                                             bsth[:], start=False, stop=True)
                            nc.vector.tensor_copy(
                                theta_r[:, 128 * h:128 * (h + 1)], pth[:])

                    def emit_slice(sl):
                        cols = slice(512 * sl, 512 * (sl + 1))
                        for dst, wts, bias_t in ((phiP, wltA, blt_c),
                                                 (gP, wgA, bg_c)):
                            pmm = psL.tile([D, 512], F32, tag="mm", bufs=2,
                                           name=f"pmm{sl}")
                            for j in range(4):
                                nc.tensor.matmul(
                                    pmm[:], wts[:, 128 * j:128 * (j + 1)],
                                    ltTP[j][:, cols],
                                    start=(j == 0), stop=(j == 3))
                            nc.scalar.activation(dst[:, cols], pmm[:],
                                                 AF.Identity,
                                                 bias=bias_t[:, 0:1])

                    ers = {}
                    phiRs = {}
                    for it in range(35):
                        if it % 4 == 0 and it // 4 < 8:
                            emit_slice(it // 4)
                        if it == 1:
                            emit_theta()
                        # stage A: transpose phi block m (contiguous now)
                        if it < 32:
                            m = it
                            ptp = psL.tile([128, 128], F16, tag="ptp", bufs=1,
                                           name=f"ptp{m}")
                            nc.tensor.transpose(
                                ptp[:], phiP[:, 128 * m:128 * (m + 1)],
                                identh[:])
                            phiR = lp.tile([128, 128], F16, tag="phiR", bufs=4,
                                           name=f"phiR{m}")
                            nc.vector.tensor_copy(phiR[:], ptp[:])
                            phiRs[m] = phiR
                        # stage B: attn matmul + exp
                        if 1 <= it <= 32:
                            m = it - 1
                            p_att = psL.tile([128, S], F32, tag="att2", bufs=2,
                                             name=f"patt{m}")
                            nc.tensor.matmul(p_att[:], phiRs.pop(m)[:],
                                             theta_r[:], start=True, stop=True)
                            er = lp.tile([128, S], F16, tag="er", bufs=4,
                                         name=f"er{m}")
                            nc.scalar.activation(er[:], p_att[:], AF.Exp,
                                                 scale=INV_SQRT_D)
                            ers[m] = er
                        # stage C: accumulate out2 (alternating banks) + sums
                        if it >= 3:
                            m = it - 3
                            er = ers.pop(m)
                            nc.tensor.matmul(p_acc[m % 2][:],
                                             gP[:, 128 * m:128 * (m + 1)],
                                             er[:], start=(m < 2),
                                             stop=(m >= 30))
                            nc.tensor.matmul(p_sums[:], ones_r[:], er[:],
                                             start=(m == 0), stop=(m == 31))

                    # merge banks (only one PSUM operand allowed per op)
                    uh = main.tile([D, S], F32, tag="uh")
                    nc.vector.tensor_copy(uh[:], p_acc[1][:])
                    nc.vector.tensor_tensor(u[:], p_acc[0][:], uh[:],
                                            OP.add)
                    nc.vector.tensor_copy(sums_sb[:], p_sums[:])

                # ---------- softmax-normalize + LayerNorm + ReLU ----------
                with tc.tile_pool(name="psN", bufs=1, space="PSUM") as psN:
                    # 1/sums (fast approx, ~18 bits), broadcast via PE, then
                    # one fused normalize that also emits row-sums (mean)
                    rec = main.tile([1, S], F32, tag="rec")
                    nc.vector.reciprocal_approx_fast(rec[:], sums_sb[:])
                    p_rbS = psN.tile([128, S], F32, tag="rb")
                    nc.tensor.matmul(p_rbS[:], orow_f[:], rec[:],
                                     start=True, stop=True)
                    acc2 = main.tile([128, 2], F32, tag="acc2")
                    out2 = main.tile([D, S], F32, tag="out2")
                    nc.vector.scalar_tensor_tensor(
                        out2[:], u[:], 1.0, p_rbS[:], OP.mult, OP.mult,
                        accum_out=acc2[:, 0:1])
                    sqj = main.tile([D, S], F32, tag="sqj")
                    nc.vector.scalar_tensor_tensor(
                        sqj[:], out2[:], 1.0, out2[:], OP.mult, OP.mult,
                        accum_out=acc2[:, 1:2])
                    p_st = psN.tile([1, 2], F32, tag="st")
                    nc.tensor.matmul(p_st[:], ones_f[:], acc2[:],
                                     start=True, stop=True)
                    stat = main.tile([1, 4], F32, tag="stat")
                    # mean, e2
                    nc.vector.tensor_scalar(stat[:, 0:2], p_st[:],
                                            1.0 / (D * S), None, OP.mult)
                    # var = e2 - mean^2 ; vare = var + eps
                    nc.vector.tensor_tensor(stat[:, 2:3], stat[:, 0:1],
                                            stat[:, 0:1], OP.mult)
                    nc.vector.tensor_tensor(stat[:, 3:4], stat[:, 1:2],
                                            stat[:, 2:3], OP.subtract)
                    vare = main.tile([1, 1], F32, tag="vare")
                    nc.vector.tensor_scalar(vare[:], stat[:, 3:4], LN_EPS,
                                            None, OP.add)
                    sqv = main.tile([1, 1], F32, tag="sqv")
                    nc.scalar.activation(sqv[:], vare[:], AF.Sqrt)
                    ms = main.tile([1, 2], F32, tag="ms")
                    nc.vector.tensor_copy(ms[:, 0:1], stat[:, 0:1])
                    nc.vector.reciprocal(ms[:, 1:2], sqv[:])
                    p_ms = psN.tile([128, 2], F32, tag="ms2")
                    nc.tensor.matmul(p_ms[:], orow_f[:], ms[:],
                                     start=True, stop=True)
                    # y_pre = ((out2 - m) * gamma) * r + beta, fp16; the ReLU
                    # rides the epilogue's PSUM->SBUF copies (replication is
                    # linear, so relu(ones (x) y_pre) == ones (x) relu(y_pre))
                    t2p = main.tile([D, S], F32, tag="t2p")
                    nc.vector.scalar_tensor_tensor(
                        t2p[:], out2[:], p_ms[:, 0:1], gam[:],
                        OP.subtract, OP.mult)
                    nc.vector.scalar_tensor_tensor(
                        y_h[:], t2p[:], p_ms[:, 1:2], bet[:],
                        OP.mult, OP.add)

            # ---------- epilogue: outT[k, c*256+s] = w[k]*y + b[k] ----------
            # y -> DRAM (64KB) -> back as one partition row; idle PE
            # replicates it to 128 partitions; DVE does the fused mult-add.
            nc.sync.dma_start(d_y[:, :], y_h[:])
            ybase = d_y[:, :]
            import concourse.bass as bass_mod
            yr_src = bass_mod.AP(tensor=ybase.tensor, offset=ybase.offset,
                                 ap=[[J, 1], [1, J]])
            nc.sync.dma_start(yrow[0:1, :], yr_src)
            with tc.tile_pool(name="epi", bufs=1) as ep, \
                 tc.tile_pool(name="psE", bufs=1, space="PSUM") as psE:
                off = 0
                for ci, n5 in enumerate(SIZES5):
                    sz = 512 * n5
                    ybg = ep.tile([128, 4096], F16, tag="ybg", bufs=3,
                                  name=f"ybg{ci}")
                    for t in range(n5):
                        pb = psE.tile([128, 512], F32, tag="pb", bufs=6,
                                      name=f"pb{ci}_{t}")
                        nc.tensor.matmul(
                            pb[:], orow_h[:, 0:128],
                            yrow[0:1, off + 512 * t:off + 512 * (t + 1)],
                            start=True, stop=True)
                        nc.scalar.activation(ybg[:, 512 * t:512 * (t + 1)],
                                             pb[:], AF.Relu)
                    for kb in range(4):
                        oc = ep.tile([128, 4096], F16, tag="oc", bufs=12,
                                     name=f"oc{ci}_{kb}")
                        nc.vector.tensor_scalar(
                            oc[:, 0:sz], ybg[:, 0:sz], wk[:, kb:kb + 1],
                            bk[:, kb:kb + 1], OP.mult, OP.add)
                        nc.sync.dma_start(
                            d_out[128 * kb:128 * (kb + 1), off:off + sz],
                            oc[:, 0:sz])
                    off += sz

    nc.compile()
    return nc


def _get_program():
    if "nc" not in _CACHE:
        _CACHE["nc"] = _build_program()
    return _CACHE["nc"]


def _install_ntff_shim():
    """Provide antenv.axon_hooks (absent in this image) so trace=True can
    capture NTFF profiles through the axon .so. Best-effort."""
    import sys
    import types
    try:
        from antenv.axon_hooks import get_axon_ntff_profile_hook  # noqa
        return
    except ImportError:
        pass
    try:
        from trn_agent_boot.trn_boot import _ntff_profile_via_ctypes
        hook = _ntff_profile_via_ctypes("/opt/axon/libaxon_pjrt.so")
        mod = types.ModuleType("antenv.axon_hooks")
        state = {"h": hook}
        mod.set_axon_ntff_profile_hook = lambda h: state.__setitem__("h", h)
        mod.get_axon_ntff_profile_hook = lambda: state["h"]
        sys.modules["antenv.axon_hooks"] = mod
        import antenv
        antenv.axon_hooks = mod
    except Exception as e:  # profiling is optional
        print(f"ntff shim unavailable: {e}")


def kernel(st_feat, lt_feat, w_st, b_st, w_lt, b_lt, w_g, b_g,
           ln_gamma, ln_beta, w_out, b_out):
    from concourse.bass_utils import run_bass_kernel_spmd
    global LAST_EXEC_NS

    st_feat = np.asarray(st_feat, dtype=np.float32)
    lt_feat = np.asarray(lt_feat, dtype=np.float32)

    def pack4(a):  # (512, X) -> (128, 4*X) with block j at cols [X*j, X*j+X)
        x = a.shape[1]
        return np.ascontiguousarray(
            a.reshape(4, 128, x).transpose(1, 0, 2).reshape(128, 4 * x))

    wst = pack4(np.asarray(w_st, np.float32).astype(np.float16))
    wlt = pack4(np.asarray(w_lt, np.float32).astype(np.float16))
    wg = pack4(np.asarray(w_g, np.float32).astype(np.float16))
    gam = np.ascontiguousarray(np.asarray(ln_gamma, np.float32)
                               .reshape(D, S))
    bet = np.ascontiguousarray(np.asarray(ln_beta, np.float32).reshape(D, S))
    bstv = np.asarray(b_st, np.float32).astype(np.float16).reshape(1, D)
    bltv = np.asarray(b_lt, np.float32).reshape(D, 1)
    bgv = np.asarray(b_g, np.float32).reshape(D, 1)
    identh = np.eye(128, dtype=np.float16)
    wkv = np.ascontiguousarray(
        np.asarray(w_out, np.float32).reshape(4, 128).T)
    bkv = np.ascontiguousarray(
        np.asarray(b_out, np.float32).reshape(4, 128).T)

    in_maps = []
    for n in range(NB):
        # column-permuted transposes: ltTP[c, m*128 + i] = ltT[c, 32*i + m]
        # and stTP[c, h*128 + i] = stT[c, 2*i + h]
        ltT = lt_feat[n].reshape(L, C).T.astype(np.float16)
        ltTP = np.ascontiguousarray(
            ltT.reshape(C, 128, 32).transpose(0, 2, 1).reshape(C, L))
        stT = st_feat[n].reshape(S, C).T.astype(np.float16)
        stTP = pack4(np.ascontiguousarray(
            stT.reshape(C, 128, 2).transpose(0, 2, 1).reshape(C, S)))
        in_maps.append({
            "ltT": ltTP, "stT": stTP, "wst": wst, "wlt": wlt, "wg": wg,
            "bst": bstv, "blt": bltv, "bg": bgv,
            "gam": gam, "bet": bet, "identh": identh,
            "wk": wkv, "bk": bkv,
        })

    nc = _get_program()
    trace = os.environ.get("BASS_KERNEL_TRACE", "") == "1"
    if trace:
        _install_ntff_shim()
    res = run_bass_kernel_spmd(nc, in_maps, core_ids=list(range(NB)),
                               trace=trace)
    LAST_EXEC_NS = res.exec_time_ns
    out = np.empty((NB, D, S, 1, C), np.float32)
    for n in range(NB):
        r = np.asarray(res.results[n]["out"])  # (512, 32768) fp16
        out[n] = (r.reshape(C, D, S).transpose(1, 2, 0)
                  .astype(np.float32).reshape(D, S, 1, C))
    return out
